# revision 12
# baseline (speedup 1.0000x reference)
"""MoE feed-forward Trainium2 kernel — host-routed expert-parallel with
on-device all-to-all dispatch/combine (8 cores).

The router (x @ Wr -> softmax -> top-2) runs on the host in fp32 numpy
(bit-identical top-2 vs the reference); the host also precomputes all
dispatch/combine index vectors.  Each NeuronCore owns ONE expert and its
2048-token shard of x (bf16).  On device:

  1. dispatch: indirect-gather my tokens into 8 capacity-padded per-expert
     buckets, AllToAll them so core e holds every token routed to expert e,
  2. dense 2-layer MLP in bf16 (fp32 PSUM, exact-erf Gelu), each output row
     scaled by its top-k softmax weight (0 for capacity padding),
  3. AllToAll the weighted rows back, then gather+add the two expert
     contributions per token.

Per-core kernel I/O is ~21 MB in / 4 MB out (vs 276 MB replicated fp32
weights for the data-parallel variant).  The host only casts dtypes,
computes the (cheap) router, and re-assembles the fp32 output.

Self-contained: hardcodes B=4, T=4096, D=1024, F=4096, E=8, TOP_K=2.
"""

import numpy as np
import ml_dtypes

import concourse.bacc as bacc
import concourse.bass as bass
import concourse.mybir as mybir
import concourse.tile as tile
from concourse.bass import IndirectOffsetOnAxis
from concourse.bass_utils import run_bass_kernel_spmd
from concourse.masks import make_identity

F32 = mybir.dt.float32
I32 = mybir.dt.int32
BF16 = mybir.dt.bfloat16
AF = mybir.ActivationFunctionType

B, T, D, F, E, TOP_K = 4, 4096, 1024, 4096, 8, 2
N_CORES = 8
N_TOKENS = B * T
TOK = N_TOKENS // N_CORES  # tokens per core (owner shard)
BLK = 512                  # token block (moving-dim of both matmuls)

BF = ml_dtypes.bfloat16


def build_moe_a2a(nc, sh):
    """sh = per-(core,expert) bucket capacity; cap = 8*sh slots per core."""
    cap = E * sh
    assert cap % BLK == 0
    NB = cap // BLK
    ND, NF = D // 128, F // 128
    NT = BLK // 128
    TT = TOK // 128

    w1 = nc.dram_tensor("w1", [D, F], BF16, kind="ExternalInput").ap()
    w2 = nc.dram_tensor("w2", [F, D], BF16, kind="ExternalInput").ap()
    xc = nc.dram_tensor("xc", [TOK, D], BF16, kind="ExternalInput").ap()
    sidx = nc.dram_tensor("sidx", [cap, 1], I32, kind="ExternalInput").ap()
    wc = nc.dram_tensor("wc", [cap, 1], F32, kind="ExternalInput").ap()
    cidx0 = nc.dram_tensor("cidx0", [TOK, 1], I32, kind="ExternalInput").ap()
    cidx1 = nc.dram_tensor("cidx1", [TOK, 1], I32, kind="ExternalInput").ap()
    yc = nc.dram_tensor("yc", [TOK, D], BF16, kind="ExternalOutput").ap()

    send1 = nc.dram_tensor("send1", [cap, D], BF16).ap()
    recv1 = nc.dram_tensor("recv1", [cap, D], BF16).ap()
    send2 = nc.dram_tensor("send2", [cap, D], BF16).ap()
    recv2 = nc.dram_tensor("recv2", [cap, D], BF16).ap()

    rg = [list(range(N_CORES))]

    with tile.TileContext(nc) as tc:
        with tc.tile_pool(name="wpool", bufs=1) as wp, \
             tc.tile_pool(name="sgp", bufs=2) as sgp, \
             tc.tile_pool(name="xrp", bufs=2) as xrp, \
             tc.tile_pool(name="xtp", bufs=2) as xtp, \
             tc.tile_pool(name="hp", bufs=1) as hp, \
             tc.tile_pool(name="outp", bufs=3) as outp, \
             tc.tile_pool(name="cmb", bufs=1) as cmb, \
             tc.tile_pool(name="ps1", bufs=2, space="PSUM") as ps1p, \
             tc.tile_pool(name="ps2", bufs=1, space="PSUM") as ps2p, \
             tc.tile_pool(name="pst", bufs=2, space="PSUM") as pstp:
            ident = wp.tile([128, 128], BF16, tag="ident")
            make_identity(nc, ident)

            # ---- small index/weight tables ----
            sidx_sb = wp.tile([128, cap // 128], I32, tag="sidx")
            nc.sync.dma_start(sidx_sb,
                              sidx.rearrange("(a p) one -> p (a one)", p=128))
            wc_sb = wp.tile([128, cap // 128], F32, tag="wc")
            nc.sync.dma_start(wc_sb,
                              wc.rearrange("(a p) one -> p (a one)", p=128))
            ci0_sb = wp.tile([128, TT], I32, tag="ci0")
            nc.sync.dma_start(ci0_sb,
                              cidx0.rearrange("(a p) one -> p (a one)", p=128))
            ci1_sb = wp.tile([128, TT], I32, tag="ci1")
            nc.sync.dma_start(ci1_sb,
                              cidx1.rearrange("(a p) one -> p (a one)", p=128))

            # ---- dispatch: gather my tokens into per-expert buckets ----
            for s in range(cap // 128):
                sg = sgp.tile([128, D], BF16, tag="sg")
                nc.gpsimd.indirect_dma_start(
                    out=sg, out_offset=None, in_=xc,
                    in_offset=IndirectOffsetOnAxis(
                        ap=sidx_sb[:, s:s + 1], axis=0))
                nc.sync.dma_start(send1[s * 128:(s + 1) * 128, :], sg)
            nc.gpsimd.collective_compute(
                "AllToAll", mybir.AluOpType.bypass, replica_groups=rg,
                ins=[send1], outs=[recv1])

            # ---- resident weights (DMA overlaps the dispatch A2A) ----
            w1_sb = []
            for d in range(ND):
                t_ = wp.tile([128, F], BF16, tag=f"w1_{d}", name=f"w1_{d}")
                nc.sync.dma_start(t_, w1[d * 128:(d + 1) * 128, :])
                w1_sb.append(t_)
            w2_sb = []
            for f in range(NF):
                t_ = wp.tile([128, D], BF16, tag=f"w2_{f}", name=f"w2_{f}")
                nc.sync.dma_start(t_, w2[f * 128:(f + 1) * 128, :])
                w2_sb.append(t_)

            # ---- dense expert MLP over token blocks ----
            for b in range(NB):
                xraw = []
                for q in range(NT):
                    r_ = xrp.tile([128, D], BF16, tag=f"xr{q}", name=f"xr{q}")
                    nc.sync.dma_start(
                        r_, recv1[b * BLK + q * 128:b * BLK + (q + 1) * 128, :])
                    xraw.append(r_)
                xb = []
                for d in range(ND):
                    t_ = xtp.tile([128, BLK], BF16, tag=f"xt{d}", name=f"xt{d}")
                    for q in range(NT):
                        tp = pstp.tile([128, 128], BF16, tag="tps", name="tps")
                        nc.tensor.transpose(
                            tp, xraw[q][:, d * 128:(d + 1) * 128], ident)
                        nc.vector.tensor_copy(
                            t_[:, q * 128:(q + 1) * 128], tp)
                    xb.append(t_)
                hb = []
                for f in range(NF):
                    ps = ps1p.tile([128, BLK], F32, tag="mm1ps", name="mm1ps")
                    for d in range(ND):
                        nc.tensor.matmul(
                            ps, w1_sb[d][:, f * 128:(f + 1) * 128], xb[d],
                            start=(d == 0), stop=(d == ND - 1))
                    h_ = hp.tile([128, BLK], BF16, tag=f"h{f}", name=f"h{f}")
                    nc.scalar.activation(h_, ps, AF.Gelu)
                    hb.append(h_)
                for dch in range(2):
                    pys = [ps2p.tile([128, 512], F32, tag=f"py{t}", name=f"py{t}")
                           for t in range(NT)]
                    for f in range(NF):
                        for t in range(NT):
                            nc.tensor.matmul(
                                pys[t],
                                hb[f][:, t * 128:(t + 1) * 128],
                                w2_sb[f][:, dch * 512:(dch + 1) * 512],
                                start=(f == 0), stop=(f == NF - 1))
                    for t in range(NT):
                        yt = outp.tile([128, 512], BF16, tag="yt")
                        nc.vector.tensor_scalar_mul(
                            yt, pys[t], wc_sb[:, b * NT + t:b * NT + t + 1])
                        nc.sync.dma_start(
                            send2[b * BLK + t * 128:b * BLK + (t + 1) * 128,
                                  dch * 512:(dch + 1) * 512], yt)

            # ---- combine: return rows to owners, add the two experts ----
            nc.gpsimd.collective_compute(
                "AllToAll", mybir.AluOpType.bypass, replica_groups=rg,
                ins=[send2], outs=[recv2])
            for t in range(TT):
                gA = cmb.tile([128, D], BF16, tag="gA")
                nc.gpsimd.indirect_dma_start(
                    out=gA, out_offset=None, in_=recv2,
                    in_offset=IndirectOffsetOnAxis(ap=ci0_sb[:, t:t + 1], axis=0))
                gB = cmb.tile([128, D], BF16, tag="gB")
                nc.gpsimd.indirect_dma_start(
                    out=gB, out_offset=None, in_=recv2,
                    in_offset=IndirectOffsetOnAxis(ap=ci1_sb[:, t:t + 1], axis=0))
                yo = cmb.tile([128, D], BF16, tag="yo")
                nc.vector.tensor_add(yo, gA, gB)
                nc.sync.dma_start(yc[t * 128:(t + 1) * 128, :], yo)
    return nc


_COMPILED = {}


def _get_compiled(sh):
    if sh not in _COMPILED:
        nc = bacc.Bacc("TRN2", target_bir_lowering=False, debug=False,
                       num_devices=N_CORES)
        build_moe_a2a(nc, sh)
        nc.compile()
        _COMPILED[sh] = nc
    return _COMPILED[sh]


def _route(xf, Wr):
    """fp32 router identical to the reference math: softmax + top-2."""
    logits = xf @ Wr
    ex = np.exp(logits - logits.max(axis=1, keepdims=True))
    probs = ex / ex.sum(axis=1, keepdims=True)
    part = np.argpartition(-logits, 2, axis=1)[:, :3]
    lv = np.take_along_axis(logits, part, axis=1)
    order = np.argsort(-lv, axis=1, kind="stable")
    top2 = np.take_along_axis(part, order, axis=1)[:, :2]
    wts = np.take_along_axis(probs, top2, axis=1)
    return top2, wts


def kernel(x, Wr, W1, W2, _trace=False, _tmpdir=None):
    x = np.asarray(x, dtype=np.float32)
    Wr = np.asarray(Wr, dtype=np.float32)
    W1 = np.asarray(W1, dtype=np.float32)
    W2 = np.asarray(W2, dtype=np.float32)
    xf = np.ascontiguousarray(x.reshape(N_TOKENS, D))

    top2, wts = _route(xf, Wr)

    # per-(core, expert) bucket capacity, 64-aligned so cap = 8*sh is
    # 512-aligned
    counts = np.zeros((N_CORES, E), np.int64)
    for c in range(N_CORES):
        t2 = top2[c * TOK:(c + 1) * TOK]
        counts[c] = np.bincount(t2.reshape(-1), minlength=E)
    sh = int(-(-counts.max() // 64) * 64)
    cap = E * sh

    xf_bf = xf.astype(BF)
    in_maps = []
    # send order bookkeeping for the combine indices
    ranks = np.zeros((N_TOKENS, TOP_K), np.int64)  # rank within (owner, e)
    for c in range(N_CORES):
        lo = c * TOK
        t2 = top2[lo:lo + TOK]
        sidx = np.zeros((cap, 1), np.int32)
        for e in range(E):
            loc = np.flatnonzero((t2 == e).any(axis=1))
            sidx[e * sh:e * sh + len(loc), 0] = loc
            k = np.where(t2[loc, 0] == e, 0, 1)
            ranks[lo + loc, 0] = np.where(k == 0, np.arange(len(loc)),
                                          ranks[lo + loc, 0])
            ranks[lo + loc, 1] = np.where(k == 1, np.arange(len(loc)),
                                          ranks[lo + loc, 1])
        cidx = top2[lo:lo + TOK] * sh + ranks[lo:lo + TOK]  # [TOK, 2]
        in_maps.append({
            "w1": np.ascontiguousarray(W1[c].astype(BF)),
            "w2": np.ascontiguousarray(W2[c].astype(BF)),
            "xc": np.ascontiguousarray(xf_bf[lo:lo + TOK]),
            "sidx": sidx,
            "cidx0": np.ascontiguousarray(cidx[:, 0:1].astype(np.int32)),
            "cidx1": np.ascontiguousarray(cidx[:, 1:2].astype(np.int32)),
        })
    # wc for core e: shard i = weights of core i's tokens routed to e
    for e in range(E):
        wcol = np.zeros((cap, 1), np.float32)
        for c in range(N_CORES):
            lo = c * TOK
            t2 = top2[lo:lo + TOK]
            loc = np.flatnonzero((t2 == e).any(axis=1))
            k = np.where(t2[loc, 0] == e, 0, 1)
            wcol[c * sh:c * sh + len(loc), 0] = wts[lo + loc, k]
        in_maps[e]["wc"] = wcol

    nc = _get_compiled(sh)
    res = run_bass_kernel_spmd(nc, in_maps, core_ids=list(range(N_CORES)),
                               trace=_trace, tmpdir=_tmpdir)

    out = np.concatenate(
        [res.results[c]["yc"].astype(np.float32) for c in range(N_CORES)],
        axis=0)
    full = out.reshape(B, T, D)
    if _trace:
        return full, res
    return full


# revision 13
# speedup vs baseline: 1.0276x; 1.0276x over previous
"""MoE feed-forward Trainium2 kernel — host-routed expert-parallel with
on-device all-to-all dispatch/combine (8 cores).

The router (x @ Wr -> softmax -> top-2) runs on the host in fp32 numpy
(bit-identical top-2 vs the reference); the host also precomputes all
dispatch/combine index vectors.  Each NeuronCore owns ONE expert and its
2048-token shard of x (bf16).  On device:

  1. dispatch: indirect-gather my tokens into 8 capacity-padded per-expert
     buckets, AllToAll them so core e holds every token routed to expert e,
  2. dense 2-layer MLP in bf16 (fp32 PSUM, exact-erf Gelu), each output row
     scaled by its top-k softmax weight (0 for capacity padding),
  3. AllToAll the weighted rows back, then gather+add the two expert
     contributions per token.

Per-core kernel I/O is ~21 MB in / 4 MB out (vs 276 MB replicated fp32
weights for the data-parallel variant).  The host only casts dtypes,
computes the (cheap) router, and re-assembles the fp32 output.

Self-contained: hardcodes B=4, T=4096, D=1024, F=4096, E=8, TOP_K=2.
"""

import numpy as np
import ml_dtypes

import concourse.bacc as bacc
import concourse.bass as bass
import concourse.mybir as mybir
import concourse.tile as tile
from concourse.bass import IndirectOffsetOnAxis
from concourse.bass_utils import run_bass_kernel_spmd
from concourse.masks import make_identity

F32 = mybir.dt.float32
I32 = mybir.dt.int32
BF16 = mybir.dt.bfloat16
AF = mybir.ActivationFunctionType

B, T, D, F, E, TOP_K = 4, 4096, 1024, 4096, 8, 2
N_CORES = 8
N_TOKENS = B * T
TOK = N_TOKENS // N_CORES  # tokens per core (owner shard)
BLK = 512                  # token block (moving-dim of both matmuls)

BF = ml_dtypes.bfloat16


def build_moe_a2a(nc, sh):
    """sh = per-(core,expert) bucket capacity; cap = 8*sh slots per core."""
    cap = E * sh
    assert cap % BLK == 0
    NB = cap // BLK
    ND, NF = D // 128, F // 128
    NT = BLK // 128
    TT = TOK // 128

    w1 = nc.dram_tensor("w1", [D, F], BF16, kind="ExternalInput").ap()
    w2 = nc.dram_tensor("w2", [F, D], BF16, kind="ExternalInput").ap()
    xc = nc.dram_tensor("xc", [TOK, D], BF16, kind="ExternalInput").ap()
    sidx = nc.dram_tensor("sidx", [cap, 1], I32, kind="ExternalInput").ap()
    wc = nc.dram_tensor("wc", [cap, 1], F32, kind="ExternalInput").ap()
    cidx0 = nc.dram_tensor("cidx0", [TOK, 1], I32, kind="ExternalInput").ap()
    cidx1 = nc.dram_tensor("cidx1", [TOK, 1], I32, kind="ExternalInput").ap()
    yc = nc.dram_tensor("yc", [TOK, D], BF16, kind="ExternalOutput").ap()

    send1 = nc.dram_tensor("send1", [cap, D], BF16).ap()
    recv1 = nc.dram_tensor("recv1", [cap, D], BF16).ap()
    send2 = nc.dram_tensor("send2", [cap, D], BF16).ap()
    recv2 = nc.dram_tensor("recv2", [cap, D], BF16).ap()

    rg = [list(range(N_CORES))]

    with tile.TileContext(nc) as tc:
        with tc.tile_pool(name="wpool", bufs=1) as wp, \
             tc.tile_pool(name="sgp", bufs=2) as sgp, \
             tc.tile_pool(name="xrp", bufs=2) as xrp, \
             tc.tile_pool(name="xtp", bufs=2) as xtp, \
             tc.tile_pool(name="hp", bufs=1) as hp, \
             tc.tile_pool(name="outp", bufs=3) as outp, \
             tc.tile_pool(name="cmb", bufs=1) as cmb, \
             tc.tile_pool(name="ps1", bufs=2, space="PSUM") as ps1p, \
             tc.tile_pool(name="ps2", bufs=1, space="PSUM") as ps2p, \
             tc.tile_pool(name="pst", bufs=2, space="PSUM") as pstp:
            ident = wp.tile([128, 128], BF16, tag="ident")
            make_identity(nc, ident)

            # ---- small index/weight tables ----
            sidx_sb = wp.tile([128, cap // 128], I32, tag="sidx")
            nc.sync.dma_start(sidx_sb,
                              sidx.rearrange("(a p) one -> p (a one)", p=128))
            wc_sb = wp.tile([128, cap // 128], F32, tag="wc")
            nc.sync.dma_start(wc_sb,
                              wc.rearrange("(a p) one -> p (a one)", p=128))
            ci0_sb = wp.tile([128, TT], I32, tag="ci0")
            nc.sync.dma_start(ci0_sb,
                              cidx0.rearrange("(a p) one -> p (a one)", p=128))
            ci1_sb = wp.tile([128, TT], I32, tag="ci1")
            nc.sync.dma_start(ci1_sb,
                              cidx1.rearrange("(a p) one -> p (a one)", p=128))

            # ---- dispatch: gather my tokens into per-expert buckets ----
            for s in range(cap // 128):
                sg = sgp.tile([128, D], BF16, tag="sg")
                nc.gpsimd.indirect_dma_start(
                    out=sg, out_offset=None, in_=xc,
                    in_offset=IndirectOffsetOnAxis(
                        ap=sidx_sb[:, s:s + 1], axis=0))
                nc.sync.dma_start(send1[s * 128:(s + 1) * 128, :], sg)
            nc.gpsimd.collective_compute(
                "AllToAll", mybir.AluOpType.bypass, replica_groups=rg,
                ins=[send1], outs=[recv1])

            # ---- resident weights (DMA overlaps the dispatch A2A) ----
            w1_sb = []
            for d in range(ND):
                t_ = wp.tile([128, F], BF16, tag=f"w1_{d}", name=f"w1_{d}")
                nc.sync.dma_start(t_, w1[d * 128:(d + 1) * 128, :])
                w1_sb.append(t_)
            w2_sb = []
            for f in range(NF):
                t_ = wp.tile([128, D], BF16, tag=f"w2_{f}", name=f"w2_{f}")
                nc.sync.dma_start(t_, w2[f * 128:(f + 1) * 128, :])
                w2_sb.append(t_)

            # ---- dense expert MLP over token blocks ----
            for b in range(NB):
                xb = []
                for d in range(ND):
                    t_ = xtp.tile([128, BLK], BF16, tag=f"xt{d}", name=f"xt{d}")
                    nc.sync.dma_start(
                        t_,
                        recv1[b * BLK:(b + 1) * BLK, d * 128:(d + 1) * 128],
                        transpose=True)
                    xb.append(t_)
                hb = []
                for f in range(NF):
                    ps = ps1p.tile([128, BLK], F32, tag="mm1ps", name="mm1ps")
                    for d in range(ND):
                        nc.tensor.matmul(
                            ps, w1_sb[d][:, f * 128:(f + 1) * 128], xb[d],
                            start=(d == 0), stop=(d == ND - 1))
                    h_ = hp.tile([128, BLK], BF16, tag=f"h{f}", name=f"h{f}")
                    nc.scalar.activation(h_, ps, AF.Gelu)
                    hb.append(h_)
                for dch in range(2):
                    pys = [ps2p.tile([128, 512], F32, tag=f"py{t}", name=f"py{t}")
                           for t in range(NT)]
                    for f in range(NF):
                        for t in range(NT):
                            nc.tensor.matmul(
                                pys[t],
                                hb[f][:, t * 128:(t + 1) * 128],
                                w2_sb[f][:, dch * 512:(dch + 1) * 512],
                                start=(f == 0), stop=(f == NF - 1))
                    for t in range(NT):
                        yt = outp.tile([128, 512], BF16, tag="yt")
                        nc.vector.tensor_scalar_mul(
                            yt, pys[t], wc_sb[:, b * NT + t:b * NT + t + 1])
                        nc.sync.dma_start(
                            send2[b * BLK + t * 128:b * BLK + (t + 1) * 128,
                                  dch * 512:(dch + 1) * 512], yt)

            # ---- combine: return rows to owners, add the two experts ----
            nc.gpsimd.collective_compute(
                "AllToAll", mybir.AluOpType.bypass, replica_groups=rg,
                ins=[send2], outs=[recv2])
            for t in range(TT):
                gA = cmb.tile([128, D], BF16, tag="gA")
                nc.gpsimd.indirect_dma_start(
                    out=gA, out_offset=None, in_=recv2,
                    in_offset=IndirectOffsetOnAxis(ap=ci0_sb[:, t:t + 1], axis=0))
                gB = cmb.tile([128, D], BF16, tag="gB")
                nc.gpsimd.indirect_dma_start(
                    out=gB, out_offset=None, in_=recv2,
                    in_offset=IndirectOffsetOnAxis(ap=ci1_sb[:, t:t + 1], axis=0))
                yo = cmb.tile([128, D], BF16, tag="yo")
                nc.vector.tensor_add(yo, gA, gB)
                nc.sync.dma_start(yc[t * 128:(t + 1) * 128, :], yo)
    return nc


_COMPILED = {}


def _get_compiled(sh):
    if sh not in _COMPILED:
        nc = bacc.Bacc("TRN2", target_bir_lowering=False, debug=False,
                       num_devices=N_CORES)
        build_moe_a2a(nc, sh)
        nc.compile()
        _COMPILED[sh] = nc
    return _COMPILED[sh]


def _route(xf, Wr):
    """fp32 router identical to the reference math: softmax + top-2."""
    logits = xf @ Wr
    ex = np.exp(logits - logits.max(axis=1, keepdims=True))
    probs = ex / ex.sum(axis=1, keepdims=True)
    part = np.argpartition(-logits, 2, axis=1)[:, :3]
    lv = np.take_along_axis(logits, part, axis=1)
    order = np.argsort(-lv, axis=1, kind="stable")
    top2 = np.take_along_axis(part, order, axis=1)[:, :2]
    wts = np.take_along_axis(probs, top2, axis=1)
    return top2, wts


def kernel(x, Wr, W1, W2, _trace=False, _tmpdir=None):
    x = np.asarray(x, dtype=np.float32)
    Wr = np.asarray(Wr, dtype=np.float32)
    W1 = np.asarray(W1, dtype=np.float32)
    W2 = np.asarray(W2, dtype=np.float32)
    xf = np.ascontiguousarray(x.reshape(N_TOKENS, D))

    top2, wts = _route(xf, Wr)

    # per-(core, expert) bucket capacity, 64-aligned so cap = 8*sh is
    # 512-aligned
    counts = np.zeros((N_CORES, E), np.int64)
    for c in range(N_CORES):
        t2 = top2[c * TOK:(c + 1) * TOK]
        counts[c] = np.bincount(t2.reshape(-1), minlength=E)
    sh = int(-(-counts.max() // 64) * 64)
    cap = E * sh

    xf_bf = xf.astype(BF)
    in_maps = []
    # send order bookkeeping for the combine indices
    ranks = np.zeros((N_TOKENS, TOP_K), np.int64)  # rank within (owner, e)
    for c in range(N_CORES):
        lo = c * TOK
        t2 = top2[lo:lo + TOK]
        sidx = np.zeros((cap, 1), np.int32)
        for e in range(E):
            loc = np.flatnonzero((t2 == e).any(axis=1))
            sidx[e * sh:e * sh + len(loc), 0] = loc
            k = np.where(t2[loc, 0] == e, 0, 1)
            ranks[lo + loc, 0] = np.where(k == 0, np.arange(len(loc)),
                                          ranks[lo + loc, 0])
            ranks[lo + loc, 1] = np.where(k == 1, np.arange(len(loc)),
                                          ranks[lo + loc, 1])
        cidx = top2[lo:lo + TOK] * sh + ranks[lo:lo + TOK]  # [TOK, 2]
        in_maps.append({
            "w1": np.ascontiguousarray(W1[c].astype(BF)),
            "w2": np.ascontiguousarray(W2[c].astype(BF)),
            "xc": np.ascontiguousarray(xf_bf[lo:lo + TOK]),
            "sidx": sidx,
            "cidx0": np.ascontiguousarray(cidx[:, 0:1].astype(np.int32)),
            "cidx1": np.ascontiguousarray(cidx[:, 1:2].astype(np.int32)),
        })
    # wc for core e: shard i = weights of core i's tokens routed to e
    for e in range(E):
        wcol = np.zeros((cap, 1), np.float32)
        for c in range(N_CORES):
            lo = c * TOK
            t2 = top2[lo:lo + TOK]
            loc = np.flatnonzero((t2 == e).any(axis=1))
            k = np.where(t2[loc, 0] == e, 0, 1)
            wcol[c * sh:c * sh + len(loc), 0] = wts[lo + loc, k]
        in_maps[e]["wc"] = wcol

    nc = _get_compiled(sh)
    res = run_bass_kernel_spmd(nc, in_maps, core_ids=list(range(N_CORES)),
                               trace=_trace, tmpdir=_tmpdir)

    out = np.concatenate(
        [res.results[c]["yc"].astype(np.float32) for c in range(N_CORES)],
        axis=0)
    full = out.reshape(B, T, D)
    if _trace:
        return full, res
    return full


# revision 14
# speedup vs baseline: 1.0318x; 1.0041x over previous
"""MoE feed-forward Trainium2 kernel — host-routed expert-parallel with
on-device all-to-all dispatch/combine (8 cores).

The router (x @ Wr -> softmax -> top-2) runs on the host in fp32 numpy
(bit-identical top-2 vs the reference); the host also precomputes all
dispatch/combine index vectors.  Each NeuronCore owns ONE expert and its
2048-token shard of x (bf16).  On device:

  1. dispatch: indirect-gather my tokens into 8 capacity-padded per-expert
     buckets, AllToAll them so core e holds every token routed to expert e,
  2. dense 2-layer MLP in bf16 (fp32 PSUM, exact-erf Gelu), each output row
     scaled by its top-k softmax weight (0 for capacity padding),
  3. AllToAll the weighted rows back, then gather+add the two expert
     contributions per token.

Per-core kernel I/O is ~21 MB in / 4 MB out (vs 276 MB replicated fp32
weights for the data-parallel variant).  The host only casts dtypes,
computes the (cheap) router, and re-assembles the fp32 output.

Self-contained: hardcodes B=4, T=4096, D=1024, F=4096, E=8, TOP_K=2.
"""

import numpy as np
import ml_dtypes

import concourse.bacc as bacc
import concourse.bass as bass
import concourse.mybir as mybir
import concourse.tile as tile
from concourse.bass import IndirectOffsetOnAxis
from concourse.bass_utils import run_bass_kernel_spmd
from concourse.masks import make_identity

F32 = mybir.dt.float32
I32 = mybir.dt.int32
BF16 = mybir.dt.bfloat16
AF = mybir.ActivationFunctionType

B, T, D, F, E, TOP_K = 4, 4096, 1024, 4096, 8, 2
N_CORES = 8
N_TOKENS = B * T
TOK = N_TOKENS // N_CORES  # tokens per core (owner shard)
BLK = 512                  # token block (moving-dim of both matmuls)

BF = ml_dtypes.bfloat16


def build_moe_a2a(nc, sh):
    """sh = per-(core,expert) bucket capacity; cap = 8*sh slots per core."""
    cap = E * sh
    assert cap % BLK == 0
    NB = cap // BLK
    ND, NF = D // 128, F // 128
    NT = BLK // 128
    TT = TOK // 128

    w1 = nc.dram_tensor("w1", [D, F], BF16, kind="ExternalInput").ap()
    w2 = nc.dram_tensor("w2", [F, D], BF16, kind="ExternalInput").ap()
    xc = nc.dram_tensor("xc", [TOK, D], BF16, kind="ExternalInput").ap()
    sidx = nc.dram_tensor("sidx", [cap, 1], I32, kind="ExternalInput").ap()
    wc = nc.dram_tensor("wc", [cap, 1], F32, kind="ExternalInput").ap()
    cidx0 = nc.dram_tensor("cidx0", [TOK, 1], I32, kind="ExternalInput").ap()
    cidx1 = nc.dram_tensor("cidx1", [TOK, 1], I32, kind="ExternalInput").ap()
    yc = nc.dram_tensor("yc", [TOK, D], BF16, kind="ExternalOutput").ap()

    send1 = nc.dram_tensor("send1", [cap, D], BF16).ap()
    recv1 = nc.dram_tensor("recv1", [cap, D], BF16).ap()
    send2 = nc.dram_tensor("send2", [cap, D], BF16).ap()
    recv2 = nc.dram_tensor("recv2", [cap, D], BF16).ap()

    rg = [list(range(N_CORES))]

    with tile.TileContext(nc) as tc:
        with tc.tile_pool(name="wpool", bufs=1) as wp, \
             tc.tile_pool(name="sgp", bufs=2) as sgp, \
             tc.tile_pool(name="xrp", bufs=2) as xrp, \
             tc.tile_pool(name="xtp", bufs=2) as xtp, \
             tc.tile_pool(name="hp", bufs=1) as hp, \
             tc.tile_pool(name="outp", bufs=3) as outp, \
             tc.tile_pool(name="cmb", bufs=1) as cmb, \
             tc.tile_pool(name="ps1", bufs=2, space="PSUM") as ps1p, \
             tc.tile_pool(name="ps2", bufs=1, space="PSUM") as ps2p, \
             tc.tile_pool(name="pst", bufs=2, space="PSUM") as pstp:
            ident = wp.tile([128, 128], BF16, tag="ident")
            make_identity(nc, ident)

            # ---- small index/weight tables ----
            sidx_sb = wp.tile([128, cap // 128], I32, tag="sidx")
            nc.sync.dma_start(sidx_sb,
                              sidx.rearrange("(a p) one -> p (a one)", p=128))
            wc_sb = wp.tile([128, cap // 128], F32, tag="wc")
            nc.sync.dma_start(wc_sb,
                              wc.rearrange("(a p) one -> p (a one)", p=128))
            ci0_sb = wp.tile([128, TT], I32, tag="ci0")
            nc.sync.dma_start(ci0_sb,
                              cidx0.rearrange("(a p) one -> p (a one)", p=128))
            ci1_sb = wp.tile([128, TT], I32, tag="ci1")
            nc.sync.dma_start(ci1_sb,
                              cidx1.rearrange("(a p) one -> p (a one)", p=128))

            # ---- dispatch: gather my tokens into per-expert buckets ----
            for s in range(cap // 128):
                sg = sgp.tile([128, D], BF16, tag="sg")
                nc.gpsimd.indirect_dma_start(
                    out=sg, out_offset=None, in_=xc,
                    in_offset=IndirectOffsetOnAxis(
                        ap=sidx_sb[:, s:s + 1], axis=0))
                nc.sync.dma_start(send1[s * 128:(s + 1) * 128, :], sg)
            nc.gpsimd.collective_compute(
                "AllToAll", mybir.AluOpType.bypass, replica_groups=rg,
                ins=[send1], outs=[recv1])

            # ---- resident weights (DMA overlaps the dispatch A2A) ----
            w1_sb = []
            for d in range(ND):
                t_ = wp.tile([128, F], BF16, tag=f"w1_{d}", name=f"w1_{d}")
                nc.sync.dma_start(t_, w1[d * 128:(d + 1) * 128, :])
                w1_sb.append(t_)
            w2_sb = []
            for f in range(NF):
                t_ = wp.tile([128, D], BF16, tag=f"w2_{f}", name=f"w2_{f}")
                nc.sync.dma_start(t_, w2[f * 128:(f + 1) * 128, :])
                w2_sb.append(t_)

            # ---- dense expert MLP over token blocks ----
            for b in range(NB):
                xb = []
                for d in range(ND):
                    t_ = xtp.tile([128, BLK], BF16, tag=f"xt{d}", name=f"xt{d}")
                    nc.sync.dma_start(
                        t_,
                        recv1[b * BLK:(b + 1) * BLK, d * 128:(d + 1) * 128],
                        transpose=True)
                    xb.append(t_)
                hb = []
                for f in range(NF):
                    ps = ps1p.tile([128, BLK], F32, tag="mm1ps", name="mm1ps")
                    for d in range(ND):
                        nc.tensor.matmul(
                            ps, w1_sb[d][:, f * 128:(f + 1) * 128], xb[d],
                            start=(d == 0), stop=(d == ND - 1))
                    h_ = hp.tile([128, BLK], BF16, tag=f"h{f}", name=f"h{f}")
                    nc.scalar.activation(h_, ps, AF.Gelu)
                    hb.append(h_)
                # mm2 in half-blocks of 2 token-tiles; both d-chunks run
                # back-to-back on the same stationary h tile (one LDWEIGHTS
                # per two matmuls)
                for g in range(NT // 2):
                    pys = [ps2p.tile([128, 512], F32, tag=f"py{t}{dch}",
                                     name=f"py{t}{dch}")
                           for t in range(2) for dch in range(2)]
                    for f in range(NF):
                        for t in range(2):
                            tt = g * 2 + t
                            for dch in range(2):
                                nc.tensor.matmul(
                                    pys[t * 2 + dch],
                                    hb[f][:, tt * 128:(tt + 1) * 128],
                                    w2_sb[f][:, dch * 512:(dch + 1) * 512],
                                    start=(f == 0), stop=(f == NF - 1))
                    for t in range(2):
                        tt = g * 2 + t
                        for dch in range(2):
                            yt = outp.tile([128, 512], BF16, tag="yt")
                            nc.vector.tensor_scalar_mul(
                                yt, pys[t * 2 + dch],
                                wc_sb[:, b * NT + tt:b * NT + tt + 1])
                            nc.sync.dma_start(
                                send2[b * BLK + tt * 128:
                                      b * BLK + (tt + 1) * 128,
                                      dch * 512:(dch + 1) * 512], yt)

            # ---- combine: return rows to owners, add the two experts ----
            nc.gpsimd.collective_compute(
                "AllToAll", mybir.AluOpType.bypass, replica_groups=rg,
                ins=[send2], outs=[recv2])
            for t in range(TT):
                gA = cmb.tile([128, D], BF16, tag="gA")
                nc.gpsimd.indirect_dma_start(
                    out=gA, out_offset=None, in_=recv2,
                    in_offset=IndirectOffsetOnAxis(ap=ci0_sb[:, t:t + 1], axis=0))
                gB = cmb.tile([128, D], BF16, tag="gB")
                nc.gpsimd.indirect_dma_start(
                    out=gB, out_offset=None, in_=recv2,
                    in_offset=IndirectOffsetOnAxis(ap=ci1_sb[:, t:t + 1], axis=0))
                yo = cmb.tile([128, D], BF16, tag="yo")
                nc.vector.tensor_add(yo, gA, gB)
                nc.sync.dma_start(yc[t * 128:(t + 1) * 128, :], yo)
    return nc


_COMPILED = {}


def _get_compiled(sh):
    if sh not in _COMPILED:
        nc = bacc.Bacc("TRN2", target_bir_lowering=False, debug=False,
                       num_devices=N_CORES)
        build_moe_a2a(nc, sh)
        nc.compile()
        _COMPILED[sh] = nc
    return _COMPILED[sh]


def _route(xf, Wr):
    """fp32 router identical to the reference math: softmax + top-2."""
    logits = xf @ Wr
    ex = np.exp(logits - logits.max(axis=1, keepdims=True))
    probs = ex / ex.sum(axis=1, keepdims=True)
    part = np.argpartition(-logits, 2, axis=1)[:, :3]
    lv = np.take_along_axis(logits, part, axis=1)
    order = np.argsort(-lv, axis=1, kind="stable")
    top2 = np.take_along_axis(part, order, axis=1)[:, :2]
    wts = np.take_along_axis(probs, top2, axis=1)
    return top2, wts


def kernel(x, Wr, W1, W2, _trace=False, _tmpdir=None):
    x = np.asarray(x, dtype=np.float32)
    Wr = np.asarray(Wr, dtype=np.float32)
    W1 = np.asarray(W1, dtype=np.float32)
    W2 = np.asarray(W2, dtype=np.float32)
    xf = np.ascontiguousarray(x.reshape(N_TOKENS, D))

    top2, wts = _route(xf, Wr)

    # per-(core, expert) bucket capacity, 64-aligned so cap = 8*sh is
    # 512-aligned
    counts = np.zeros((N_CORES, E), np.int64)
    for c in range(N_CORES):
        t2 = top2[c * TOK:(c + 1) * TOK]
        counts[c] = np.bincount(t2.reshape(-1), minlength=E)
    sh = int(-(-counts.max() // 64) * 64)
    cap = E * sh

    xf_bf = xf.astype(BF)
    in_maps = []
    # send order bookkeeping for the combine indices
    ranks = np.zeros((N_TOKENS, TOP_K), np.int64)  # rank within (owner, e)
    for c in range(N_CORES):
        lo = c * TOK
        t2 = top2[lo:lo + TOK]
        sidx = np.zeros((cap, 1), np.int32)
        for e in range(E):
            loc = np.flatnonzero((t2 == e).any(axis=1))
            sidx[e * sh:e * sh + len(loc), 0] = loc
            k = np.where(t2[loc, 0] == e, 0, 1)
            ranks[lo + loc, 0] = np.where(k == 0, np.arange(len(loc)),
                                          ranks[lo + loc, 0])
            ranks[lo + loc, 1] = np.where(k == 1, np.arange(len(loc)),
                                          ranks[lo + loc, 1])
        cidx = top2[lo:lo + TOK] * sh + ranks[lo:lo + TOK]  # [TOK, 2]
        in_maps.append({
            "w1": np.ascontiguousarray(W1[c].astype(BF)),
            "w2": np.ascontiguousarray(W2[c].astype(BF)),
            "xc": np.ascontiguousarray(xf_bf[lo:lo + TOK]),
            "sidx": sidx,
            "cidx0": np.ascontiguousarray(cidx[:, 0:1].astype(np.int32)),
            "cidx1": np.ascontiguousarray(cidx[:, 1:2].astype(np.int32)),
        })
    # wc for core e: shard i = weights of core i's tokens routed to e
    for e in range(E):
        wcol = np.zeros((cap, 1), np.float32)
        for c in range(N_CORES):
            lo = c * TOK
            t2 = top2[lo:lo + TOK]
            loc = np.flatnonzero((t2 == e).any(axis=1))
            k = np.where(t2[loc, 0] == e, 0, 1)
            wcol[c * sh:c * sh + len(loc), 0] = wts[lo + loc, k]
        in_maps[e]["wc"] = wcol

    nc = _get_compiled(sh)
    res = run_bass_kernel_spmd(nc, in_maps, core_ids=list(range(N_CORES)),
                               trace=_trace, tmpdir=_tmpdir)

    out = np.concatenate(
        [res.results[c]["yc"].astype(np.float32) for c in range(N_CORES)],
        axis=0)
    full = out.reshape(B, T, D)
    if _trace:
        return full, res
    return full


# revision 16
# speedup vs baseline: 1.0904x; 1.0568x over previous
"""MoE feed-forward Trainium2 kernel — host-routed expert-parallel with
on-device all-to-all dispatch/combine (8 cores).

The router (x @ Wr -> softmax -> top-2) runs on the host in fp32 numpy
(bit-identical top-2 vs the reference); the host also precomputes all
dispatch/combine index vectors.  Each NeuronCore owns ONE expert and its
2048-token shard of x (bf16).  On device:

  1. dispatch: indirect-gather my tokens into 8 capacity-padded per-expert
     buckets, AllToAll them so core e holds every token routed to expert e,
  2. dense 2-layer MLP in bf16 (fp32 PSUM, exact-erf Gelu), each output row
     scaled by its top-k softmax weight (0 for capacity padding),
  3. AllToAll the weighted rows back, then gather+add the two expert
     contributions per token.

Per-core kernel I/O is ~21 MB in / 4 MB out (vs 276 MB replicated fp32
weights for the data-parallel variant).  The host only casts dtypes,
computes the (cheap) router, and re-assembles the fp32 output.

Self-contained: hardcodes B=4, T=4096, D=1024, F=4096, E=8, TOP_K=2.
"""

import numpy as np
import ml_dtypes

import concourse.bacc as bacc
import concourse.bass as bass
import concourse.mybir as mybir
import concourse.tile as tile
from concourse.bass import IndirectOffsetOnAxis
from concourse.bass_utils import run_bass_kernel_spmd
from concourse.masks import make_identity

F32 = mybir.dt.float32
I32 = mybir.dt.int32
BF16 = mybir.dt.bfloat16
AF = mybir.ActivationFunctionType

B, T, D, F, E, TOP_K = 4, 4096, 1024, 4096, 8, 2
N_CORES = 8
N_TOKENS = B * T
TOK = N_TOKENS // N_CORES  # tokens per core (owner shard)
BLK = 512                  # token block (moving-dim of both matmuls)

BF = ml_dtypes.bfloat16


def build_moe_a2a(nc, sh):
    """sh = per-(core,expert) bucket capacity; cap = 8*sh slots per core."""
    cap = E * sh
    assert cap % BLK == 0
    NB = cap // BLK
    ND, NF = D // 128, F // 128
    NT = BLK // 128
    TT = TOK // 128

    w1 = nc.dram_tensor("w1", [D, F], BF16, kind="ExternalInput").ap()
    w2 = nc.dram_tensor("w2", [F, D], BF16, kind="ExternalInput").ap()
    xc = nc.dram_tensor("xc", [TOK, D], BF16, kind="ExternalInput").ap()
    sidx = nc.dram_tensor("sidx", [cap, 1], I32, kind="ExternalInput").ap()
    wc = nc.dram_tensor("wc", [cap, 1], F32, kind="ExternalInput").ap()
    cidx0 = nc.dram_tensor("cidx0", [TOK, 1], I32, kind="ExternalInput").ap()
    cidx1 = nc.dram_tensor("cidx1", [TOK, 1], I32, kind="ExternalInput").ap()
    yc = nc.dram_tensor("yc", [TOK, D], BF16, kind="ExternalOutput").ap()

    send1 = nc.dram_tensor("send1", [cap, D], BF16).ap()
    recv1 = nc.dram_tensor("recv1", [cap, D], BF16).ap()
    send2 = nc.dram_tensor("send2", [cap, D], BF16).ap()
    recv2 = nc.dram_tensor("recv2", [cap, D], BF16).ap()

    rg = [list(range(N_CORES))]

    with tile.TileContext(nc) as tc:
        with tc.tile_pool(name="wpool", bufs=1) as wp, \
             tc.tile_pool(name="sgp", bufs=5) as sgp, \
             tc.tile_pool(name="xtp", bufs=2) as xtp, \
             tc.tile_pool(name="hp", bufs=1) as hp, \
             tc.tile_pool(name="outp", bufs=3) as outp, \
             tc.tile_pool(name="cmb", bufs=2) as cmb, \
             tc.tile_pool(name="ps1", bufs=2, space="PSUM") as ps1p, \
             tc.tile_pool(name="ps2", bufs=1, space="PSUM") as ps2p, \
             tc.tile_pool(name="pst", bufs=2, space="PSUM") as pstp:
            ident = wp.tile([128, 128], BF16, tag="ident")
            make_identity(nc, ident)

            # ---- small index/weight tables ----
            sidx_sb = wp.tile([128, cap // 128], I32, tag="sidx")
            nc.sync.dma_start(sidx_sb,
                              sidx.rearrange("(a p) one -> p (a one)", p=128))
            wc_sb = wp.tile([128, cap // 128], F32, tag="wc")
            nc.sync.dma_start(wc_sb,
                              wc.rearrange("(a p) one -> p (a one)", p=128))
            ci0_sb = wp.tile([128, TT], I32, tag="ci0")
            nc.sync.dma_start(ci0_sb,
                              cidx0.rearrange("(a p) one -> p (a one)", p=128))
            ci1_sb = wp.tile([128, TT], I32, tag="ci1")
            nc.sync.dma_start(ci1_sb,
                              cidx1.rearrange("(a p) one -> p (a one)", p=128))

            # ---- dispatch: gather my tokens into per-expert buckets ----
            for s in range(cap // 128):
                sg = sgp.tile([128, D], BF16, tag="sg")
                nc.gpsimd.indirect_dma_start(
                    out=sg, out_offset=None, in_=xc,
                    in_offset=IndirectOffsetOnAxis(
                        ap=sidx_sb[:, s:s + 1], axis=0))
                nc.sync.dma_start(send1[s * 128:(s + 1) * 128, :], sg)
            nc.gpsimd.collective_compute(
                "AllToAll", mybir.AluOpType.bypass, replica_groups=rg,
                ins=[send1], outs=[recv1])

            # ---- resident weights (DMA overlaps the dispatch A2A) ----
            w1_sb = []
            for d in range(ND):
                t_ = wp.tile([128, F], BF16, tag=f"w1_{d}", name=f"w1_{d}")
                nc.sync.dma_start(t_, w1[d * 128:(d + 1) * 128, :])
                w1_sb.append(t_)
            w2_sb = []
            for f in range(NF):
                t_ = wp.tile([128, D], BF16, tag=f"w2_{f}", name=f"w2_{f}")
                nc.sync.dma_start(t_, w2[f * 128:(f + 1) * 128, :])
                w2_sb.append(t_)

            # ---- dense expert MLP over token blocks ----
            for b in range(NB):
                xb = []
                for d in range(ND):
                    t_ = xtp.tile([128, BLK], BF16, tag=f"xt{d}", name=f"xt{d}")
                    nc.sync.dma_start(
                        t_,
                        recv1[b * BLK:(b + 1) * BLK, d * 128:(d + 1) * 128],
                        transpose=True)
                    xb.append(t_)
                hb = []
                for f in range(NF):
                    ps = ps1p.tile([128, BLK], F32, tag="mm1ps", name="mm1ps")
                    for d in range(ND):
                        nc.tensor.matmul(
                            ps, w1_sb[d][:, f * 128:(f + 1) * 128], xb[d],
                            start=(d == 0), stop=(d == ND - 1))
                    h_ = hp.tile([128, BLK], BF16, tag=f"h{f}", name=f"h{f}")
                    nc.scalar.activation(h_, ps, AF.Gelu)
                    hb.append(h_)
                # mm2 in half-blocks of 2 token-tiles; both d-chunks run
                # back-to-back on the same stationary h tile (one LDWEIGHTS
                # per two matmuls)
                for g in range(NT // 2):
                    pys = [ps2p.tile([128, 512], F32, tag=f"py{t}{dch}",
                                     name=f"py{t}{dch}")
                           for t in range(2) for dch in range(2)]
                    for f in range(NF):
                        for t in range(2):
                            tt = g * 2 + t
                            for dch in range(2):
                                nc.tensor.matmul(
                                    pys[t * 2 + dch],
                                    hb[f][:, tt * 128:(tt + 1) * 128],
                                    w2_sb[f][:, dch * 512:(dch + 1) * 512],
                                    start=(f == 0), stop=(f == NF - 1))
                    for t in range(2):
                        tt = g * 2 + t
                        for dch in range(2):
                            yt = outp.tile([128, 512], BF16, tag="yt")
                            nc.vector.tensor_scalar_mul(
                                yt, pys[t * 2 + dch],
                                wc_sb[:, b * NT + tt:b * NT + tt + 1])
                            nc.sync.dma_start(
                                send2[b * BLK + tt * 128:
                                      b * BLK + (tt + 1) * 128,
                                      dch * 512:(dch + 1) * 512], yt)

            # ---- combine: return rows to owners, add the two experts ----
            nc.gpsimd.collective_compute(
                "AllToAll", mybir.AluOpType.bypass, replica_groups=rg,
                ins=[send2], outs=[recv2])
            for t in range(TT):
                gA = cmb.tile([128, D], BF16, tag="gA")
                nc.gpsimd.indirect_dma_start(
                    out=gA, out_offset=None, in_=recv2,
                    in_offset=IndirectOffsetOnAxis(ap=ci0_sb[:, t:t + 1], axis=0))
                gB = cmb.tile([128, D], BF16, tag="gB")
                nc.gpsimd.indirect_dma_start(
                    out=gB, out_offset=None, in_=recv2,
                    in_offset=IndirectOffsetOnAxis(ap=ci1_sb[:, t:t + 1], axis=0))
                yo = cmb.tile([128, D], BF16, tag="yo")
                nc.vector.tensor_add(yo, gA, gB)
                nc.sync.dma_start(yc[t * 128:(t + 1) * 128, :], yo)
    return nc


_COMPILED = {}


def _get_compiled(sh):
    if sh not in _COMPILED:
        nc = bacc.Bacc("TRN2", target_bir_lowering=False, debug=False,
                       num_devices=N_CORES)
        build_moe_a2a(nc, sh)
        nc.compile()
        _COMPILED[sh] = nc
    return _COMPILED[sh]


def _route(xf, Wr):
    """fp32 router identical to the reference math: softmax + top-2."""
    logits = xf @ Wr
    ex = np.exp(logits - logits.max(axis=1, keepdims=True))
    probs = ex / ex.sum(axis=1, keepdims=True)
    part = np.argpartition(-logits, 2, axis=1)[:, :3]
    lv = np.take_along_axis(logits, part, axis=1)
    order = np.argsort(-lv, axis=1, kind="stable")
    top2 = np.take_along_axis(part, order, axis=1)[:, :2]
    wts = np.take_along_axis(probs, top2, axis=1)
    return top2, wts


def kernel(x, Wr, W1, W2, _trace=False, _tmpdir=None):
    x = np.asarray(x, dtype=np.float32)
    Wr = np.asarray(Wr, dtype=np.float32)
    W1 = np.asarray(W1, dtype=np.float32)
    W2 = np.asarray(W2, dtype=np.float32)
    xf = np.ascontiguousarray(x.reshape(N_TOKENS, D))

    top2, wts = _route(xf, Wr)

    # per-(core, expert) bucket capacity, 64-aligned so cap = 8*sh is
    # 512-aligned
    counts = np.zeros((N_CORES, E), np.int64)
    for c in range(N_CORES):
        t2 = top2[c * TOK:(c + 1) * TOK]
        counts[c] = np.bincount(t2.reshape(-1), minlength=E)
    sh = int(-(-counts.max() // 64) * 64)
    cap = E * sh

    xf_bf = xf.astype(BF)
    in_maps = []
    # send order bookkeeping for the combine indices
    ranks = np.zeros((N_TOKENS, TOP_K), np.int64)  # rank within (owner, e)
    for c in range(N_CORES):
        lo = c * TOK
        t2 = top2[lo:lo + TOK]
        sidx = np.zeros((cap, 1), np.int32)
        for e in range(E):
            loc = np.flatnonzero((t2 == e).any(axis=1))
            sidx[e * sh:e * sh + len(loc), 0] = loc
            k = np.where(t2[loc, 0] == e, 0, 1)
            ranks[lo + loc, 0] = np.where(k == 0, np.arange(len(loc)),
                                          ranks[lo + loc, 0])
            ranks[lo + loc, 1] = np.where(k == 1, np.arange(len(loc)),
                                          ranks[lo + loc, 1])
        cidx = top2[lo:lo + TOK] * sh + ranks[lo:lo + TOK]  # [TOK, 2]
        in_maps.append({
            "w1": np.ascontiguousarray(W1[c].astype(BF)),
            "w2": np.ascontiguousarray(W2[c].astype(BF)),
            "xc": np.ascontiguousarray(xf_bf[lo:lo + TOK]),
            "sidx": sidx,
            "cidx0": np.ascontiguousarray(cidx[:, 0:1].astype(np.int32)),
            "cidx1": np.ascontiguousarray(cidx[:, 1:2].astype(np.int32)),
        })
    # wc for core e: shard i = weights of core i's tokens routed to e
    for e in range(E):
        wcol = np.zeros((cap, 1), np.float32)
        for c in range(N_CORES):
            lo = c * TOK
            t2 = top2[lo:lo + TOK]
            loc = np.flatnonzero((t2 == e).any(axis=1))
            k = np.where(t2[loc, 0] == e, 0, 1)
            wcol[c * sh:c * sh + len(loc), 0] = wts[lo + loc, k]
        in_maps[e]["wc"] = wcol

    nc = _get_compiled(sh)
    res = run_bass_kernel_spmd(nc, in_maps, core_ids=list(range(N_CORES)),
                               trace=_trace, tmpdir=_tmpdir)

    out = np.concatenate(
        [res.results[c]["yc"].astype(np.float32) for c in range(N_CORES)],
        axis=0)
    full = out.reshape(B, T, D)
    if _trace:
        return full, res
    return full


# revision 17
# speedup vs baseline: 1.1021x; 1.0107x over previous
"""MoE feed-forward Trainium2 kernel — host-routed expert-parallel with
on-device all-to-all dispatch/combine (8 cores).

The router (x @ Wr -> softmax -> top-2) runs on the host in fp32 numpy
(bit-identical top-2 vs the reference); the host also precomputes all
dispatch/combine index vectors.  Each NeuronCore owns ONE expert and its
2048-token shard of x (bf16).  On device:

  1. dispatch: indirect-gather my tokens into 8 capacity-padded per-expert
     buckets, AllToAll them so core e holds every token routed to expert e,
  2. dense 2-layer MLP in bf16 (fp32 PSUM, exact-erf Gelu), each output row
     scaled by its top-k softmax weight (0 for capacity padding),
  3. AllToAll the weighted rows back, then gather+add the two expert
     contributions per token.

Per-core kernel I/O is ~21 MB in / 4 MB out (vs 276 MB replicated fp32
weights for the data-parallel variant).  The host only casts dtypes,
computes the (cheap) router, and re-assembles the fp32 output.

Self-contained: hardcodes B=4, T=4096, D=1024, F=4096, E=8, TOP_K=2.
"""

import numpy as np
import ml_dtypes

import concourse.bacc as bacc
import concourse.bass as bass
import concourse.mybir as mybir
import concourse.tile as tile
from concourse.bass import IndirectOffsetOnAxis
from concourse.bass_utils import run_bass_kernel_spmd
from concourse.masks import make_identity

F32 = mybir.dt.float32
I32 = mybir.dt.int32
BF16 = mybir.dt.bfloat16
AF = mybir.ActivationFunctionType

B, T, D, F, E, TOP_K = 4, 4096, 1024, 4096, 8, 2
N_CORES = 8
N_TOKENS = B * T
TOK = N_TOKENS // N_CORES  # tokens per core (owner shard)
BLK = 512                  # token block (moving-dim of both matmuls)

BF = ml_dtypes.bfloat16


def build_moe_a2a(nc, sh):
    """sh = per-(core,expert) bucket capacity; cap = 8*sh slots per core."""
    cap = E * sh
    assert cap % BLK == 0
    NB = cap // BLK
    ND, NF = D // 128, F // 128
    NT = BLK // 128
    TT = TOK // 128

    w1 = nc.dram_tensor("w1", [D, F], BF16, kind="ExternalInput").ap()
    w2 = nc.dram_tensor("w2", [F, D], BF16, kind="ExternalInput").ap()
    xc = nc.dram_tensor("xc", [TOK, D], BF16, kind="ExternalInput").ap()
    sidx = nc.dram_tensor("sidx", [cap, 1], I32, kind="ExternalInput").ap()
    wc = nc.dram_tensor("wc", [cap, 1], F32, kind="ExternalInput").ap()
    cidx0 = nc.dram_tensor("cidx0", [TOK, 1], I32, kind="ExternalInput").ap()
    cidx1 = nc.dram_tensor("cidx1", [TOK, 1], I32, kind="ExternalInput").ap()
    yc = nc.dram_tensor("yc", [TOK, D], BF16, kind="ExternalOutput").ap()

    send1 = nc.dram_tensor("send1", [cap, D], BF16).ap()
    recv1 = nc.dram_tensor("recv1", [cap, D], BF16).ap()
    send2 = nc.dram_tensor("send2", [cap, D], BF16).ap()
    recv2 = nc.dram_tensor("recv2", [cap, D], BF16).ap()

    rg = [list(range(N_CORES))]

    with tile.TileContext(nc) as tc:
        with tc.tile_pool(name="wpool", bufs=1) as wp, \
             tc.tile_pool(name="sgp", bufs=5) as sgp, \
             tc.tile_pool(name="xtp", bufs=2) as xtp, \
             tc.tile_pool(name="hp", bufs=1) as hp, \
             tc.tile_pool(name="outp", bufs=3) as outp, \
             tc.tile_pool(name="cmb", bufs=3) as cmb, \
             tc.tile_pool(name="ps1", bufs=2, space="PSUM") as ps1p, \
             tc.tile_pool(name="ps2", bufs=1, space="PSUM") as ps2p, \
             tc.tile_pool(name="pst", bufs=2, space="PSUM") as pstp:
            ident = wp.tile([128, 128], BF16, tag="ident")
            make_identity(nc, ident)

            # ---- small index/weight tables ----
            sidx_sb = wp.tile([128, cap // 128], I32, tag="sidx")
            nc.sync.dma_start(sidx_sb,
                              sidx.rearrange("(a p) one -> p (a one)", p=128))
            wc_sb = wp.tile([128, cap // 128], F32, tag="wc")
            nc.sync.dma_start(wc_sb,
                              wc.rearrange("(a p) one -> p (a one)", p=128))
            ci0_sb = wp.tile([128, TT], I32, tag="ci0")
            nc.sync.dma_start(ci0_sb,
                              cidx0.rearrange("(a p) one -> p (a one)", p=128))
            ci1_sb = wp.tile([128, TT], I32, tag="ci1")
            nc.sync.dma_start(ci1_sb,
                              cidx1.rearrange("(a p) one -> p (a one)", p=128))

            # ---- dispatch: gather my tokens into per-expert buckets ----
            for s in range(cap // 128):
                sg = sgp.tile([128, D], BF16, tag="sg")
                nc.gpsimd.indirect_dma_start(
                    out=sg, out_offset=None, in_=xc,
                    in_offset=IndirectOffsetOnAxis(
                        ap=sidx_sb[:, s:s + 1], axis=0))
                nc.sync.dma_start(send1[s * 128:(s + 1) * 128, :], sg)
            nc.gpsimd.collective_compute(
                "AllToAll", mybir.AluOpType.bypass, replica_groups=rg,
                ins=[send1], outs=[recv1])

            # ---- resident weights (DMA overlaps the dispatch A2A) ----
            w1_sb = []
            for d in range(ND):
                t_ = wp.tile([128, F], BF16, tag=f"w1_{d}", name=f"w1_{d}")
                nc.sync.dma_start(t_, w1[d * 128:(d + 1) * 128, :])
                w1_sb.append(t_)
            w2_sb = []
            for f in range(NF):
                t_ = wp.tile([128, D], BF16, tag=f"w2_{f}", name=f"w2_{f}")
                nc.sync.dma_start(t_, w2[f * 128:(f + 1) * 128, :])
                w2_sb.append(t_)

            # ---- dense expert MLP over token blocks ----
            for b in range(NB):
                xb = []
                for d in range(ND):
                    t_ = xtp.tile([128, BLK], BF16, tag=f"xt{d}", name=f"xt{d}")
                    nc.sync.dma_start(
                        t_,
                        recv1[b * BLK:(b + 1) * BLK, d * 128:(d + 1) * 128],
                        transpose=True)
                    xb.append(t_)
                hb = []
                for f in range(NF):
                    ps = ps1p.tile([128, BLK], F32, tag="mm1ps", name="mm1ps")
                    for d in range(ND):
                        nc.tensor.matmul(
                            ps, w1_sb[d][:, f * 128:(f + 1) * 128], xb[d],
                            start=(d == 0), stop=(d == ND - 1))
                    h_ = hp.tile([128, BLK], BF16, tag=f"h{f}", name=f"h{f}")
                    nc.scalar.activation(h_, ps, AF.Gelu)
                    hb.append(h_)
                # mm2 in half-blocks of 2 token-tiles; both d-chunks run
                # back-to-back on the same stationary h tile (one LDWEIGHTS
                # per two matmuls)
                for g in range(NT // 2):
                    pys = [ps2p.tile([128, 512], F32, tag=f"py{t}{dch}",
                                     name=f"py{t}{dch}")
                           for t in range(2) for dch in range(2)]
                    for f in range(NF):
                        for t in range(2):
                            tt = g * 2 + t
                            for dch in range(2):
                                nc.tensor.matmul(
                                    pys[t * 2 + dch],
                                    hb[f][:, tt * 128:(tt + 1) * 128],
                                    w2_sb[f][:, dch * 512:(dch + 1) * 512],
                                    start=(f == 0), stop=(f == NF - 1))
                    for t in range(2):
                        tt = g * 2 + t
                        for dch in range(2):
                            yt = outp.tile([128, 512], BF16, tag="yt")
                            nc.vector.tensor_scalar_mul(
                                yt, pys[t * 2 + dch],
                                wc_sb[:, b * NT + tt:b * NT + tt + 1])
                            nc.sync.dma_start(
                                send2[b * BLK + tt * 128:
                                      b * BLK + (tt + 1) * 128,
                                      dch * 512:(dch + 1) * 512], yt)

            # ---- combine: return rows to owners, add the two experts ----
            nc.gpsimd.collective_compute(
                "AllToAll", mybir.AluOpType.bypass, replica_groups=rg,
                ins=[send2], outs=[recv2])
            for t in range(TT):
                gA = cmb.tile([128, D], BF16, tag="gA")
                nc.gpsimd.indirect_dma_start(
                    out=gA, out_offset=None, in_=recv2,
                    in_offset=IndirectOffsetOnAxis(ap=ci0_sb[:, t:t + 1], axis=0))
                gB = cmb.tile([128, D], BF16, tag="gB")
                nc.gpsimd.indirect_dma_start(
                    out=gB, out_offset=None, in_=recv2,
                    in_offset=IndirectOffsetOnAxis(ap=ci1_sb[:, t:t + 1], axis=0))
                yo = cmb.tile([128, D], BF16, tag="yo")
                nc.vector.tensor_add(yo, gA, gB)
                nc.sync.dma_start(yc[t * 128:(t + 1) * 128, :], yo)
    return nc


_COMPILED = {}


def _get_compiled(sh):
    if sh not in _COMPILED:
        nc = bacc.Bacc("TRN2", target_bir_lowering=False, debug=False,
                       num_devices=N_CORES)
        build_moe_a2a(nc, sh)
        nc.compile()
        _COMPILED[sh] = nc
    return _COMPILED[sh]


def _route(xf, Wr):
    """fp32 router identical to the reference math: softmax + top-2."""
    logits = xf @ Wr
    ex = np.exp(logits - logits.max(axis=1, keepdims=True))
    probs = ex / ex.sum(axis=1, keepdims=True)
    part = np.argpartition(-logits, 2, axis=1)[:, :3]
    lv = np.take_along_axis(logits, part, axis=1)
    order = np.argsort(-lv, axis=1, kind="stable")
    top2 = np.take_along_axis(part, order, axis=1)[:, :2]
    wts = np.take_along_axis(probs, top2, axis=1)
    return top2, wts


def kernel(x, Wr, W1, W2, _trace=False, _tmpdir=None):
    x = np.asarray(x, dtype=np.float32)
    Wr = np.asarray(Wr, dtype=np.float32)
    W1 = np.asarray(W1, dtype=np.float32)
    W2 = np.asarray(W2, dtype=np.float32)
    xf = np.ascontiguousarray(x.reshape(N_TOKENS, D))

    top2, wts = _route(xf, Wr)

    # per-(core, expert) bucket capacity, 64-aligned so cap = 8*sh is
    # 512-aligned
    counts = np.zeros((N_CORES, E), np.int64)
    for c in range(N_CORES):
        t2 = top2[c * TOK:(c + 1) * TOK]
        counts[c] = np.bincount(t2.reshape(-1), minlength=E)
    sh = int(-(-counts.max() // 64) * 64)
    cap = E * sh

    xf_bf = xf.astype(BF)
    in_maps = []
    # send order bookkeeping for the combine indices
    ranks = np.zeros((N_TOKENS, TOP_K), np.int64)  # rank within (owner, e)
    for c in range(N_CORES):
        lo = c * TOK
        t2 = top2[lo:lo + TOK]
        sidx = np.zeros((cap, 1), np.int32)
        for e in range(E):
            loc = np.flatnonzero((t2 == e).any(axis=1))
            sidx[e * sh:e * sh + len(loc), 0] = loc
            k = np.where(t2[loc, 0] == e, 0, 1)
            ranks[lo + loc, 0] = np.where(k == 0, np.arange(len(loc)),
                                          ranks[lo + loc, 0])
            ranks[lo + loc, 1] = np.where(k == 1, np.arange(len(loc)),
                                          ranks[lo + loc, 1])
        cidx = top2[lo:lo + TOK] * sh + ranks[lo:lo + TOK]  # [TOK, 2]
        in_maps.append({
            "w1": np.ascontiguousarray(W1[c].astype(BF)),
            "w2": np.ascontiguousarray(W2[c].astype(BF)),
            "xc": np.ascontiguousarray(xf_bf[lo:lo + TOK]),
            "sidx": sidx,
            "cidx0": np.ascontiguousarray(cidx[:, 0:1].astype(np.int32)),
            "cidx1": np.ascontiguousarray(cidx[:, 1:2].astype(np.int32)),
        })
    # wc for core e: shard i = weights of core i's tokens routed to e
    for e in range(E):
        wcol = np.zeros((cap, 1), np.float32)
        for c in range(N_CORES):
            lo = c * TOK
            t2 = top2[lo:lo + TOK]
            loc = np.flatnonzero((t2 == e).any(axis=1))
            k = np.where(t2[loc, 0] == e, 0, 1)
            wcol[c * sh:c * sh + len(loc), 0] = wts[lo + loc, k]
        in_maps[e]["wc"] = wcol

    nc = _get_compiled(sh)
    res = run_bass_kernel_spmd(nc, in_maps, core_ids=list(range(N_CORES)),
                               trace=_trace, tmpdir=_tmpdir)

    out = np.concatenate(
        [res.results[c]["yc"].astype(np.float32) for c in range(N_CORES)],
        axis=0)
    full = out.reshape(B, T, D)
    if _trace:
        return full, res
    return full
